# revision 4
# baseline (speedup 1.0000x reference)
"""Multi-head self-attention (B=8, N=1024, C=768, H=12, D=64) on 8 Trainium2
NeuronCores, batch-parallel (one batch element per core).

v2: single software-pipelined stream. The ACT engine's exp throughput
(~110us for 12.6M elements/core) is the phase-D roofline, so all projection
work (QKV, V, out-proj) is interleaved INTO the attention stream as PE
filler instead of running as separate phases with ACT idle:

  prologue: big partition-contiguous DMAs (x, w_qk, w_v, w_p); C(p0) = Q/K
            projection for head-pair 0.
  stream p (6 pairs): per kt-step: S^T matmuls (row-tiled K=64 halves,
            concurrent) -> ACT exp -> PV matmuls of pair p-1 (1-stream lag)
            + one filler block (remaining Q/K slices, V token-tiles).
  tail:     PV(p5) drain, norms, out-projection (PSUM-accumulated, bias
            folded in as a K=1 ones-matmul) + fp16 output DMA.

The softmax denominator rides as a ones-column in the extended V weights
(w_v has 12 zero columns; a K=1 mask matmul adds the ones), giving M=65 PV
outputs [ctx_d; den]. Normalization: reciprocal on DVE, broadcast to 128
partitions via masked K=1 ones-matmuls, one in-place multiply per pair.
No max-subtract: |S*scale| < 9 so exp < 6e3 fits fp16.
"""
import numpy as np

import concourse.bass as bass
import concourse.tile as tile
from concourse import bacc, mybir
from concourse.bass_utils import run_bass_kernel_spmd

N_CORES = 8
N = 1024          # tokens per core (batch element)
C = 768           # model dim
H = 12            # heads
D = 64            # head dim
SCALE = D ** -0.5
NT = N // 128     # 8 token tiles
CT = C // 128     # 6 feature tiles (= head pairs)
DE = D + 1        # head slot in extended V (features + denominator ones col)
VW = H * DE       # 780: extended V width
F32 = mybir.dt.float32
F32R = mybir.dt.float32r
FP16 = mybir.dt.float16
EXP = mybir.ActivationFunctionType.Exp


def _r(ap):
    return ap.bitcast(F32R)


def build():
    nc = bacc.Bacc(
        "TRN2", target_bir_lowering=False, debug=False, num_devices=N_CORES
    )
    # all big inputs are laid out host-side as [128, *] partition-contiguous
    xh_d = nc.dram_tensor("xh", [128, CT * N], FP16, kind="ExternalInput").ap()
    wqk_d = nc.dram_tensor("wqk", [128, 12 * C], FP16, kind="ExternalInput").ap()
    wv_d = nc.dram_tensor("wv", [128, CT * VW], FP16, kind="ExternalInput").ap()
    wp_d = nc.dram_tensor("wp", [128, CT * C], FP16, kind="ExternalInput").ap()
    # consts16: [vmask(780) | bias(768) | ones(128)]
    cst_d = nc.dram_tensor("cst", [1, VW + C + 128], FP16, kind="ExternalInput").ap()
    onesr_d = nc.dram_tensor("ones_mask", [2, 128], F32, kind="ExternalInput").ap()
    out_d = nc.dram_tensor("out", [N, C], FP16, kind="ExternalOutput").ap()

    with tile.TileContext(nc) as tc:
        with (
            tc.tile_pool(name="big", bufs=1) as big,
            tc.tile_pool(name="e", bufs=4) as ep,
            tc.tile_pool(name="outb", bufs=2) as outp,
            tc.tile_pool(name="norm", bufs=2) as normp,
            tc.tile_pool(name="psA", bufs=2, space="PSUM") as psA,
            tc.tile_pool(name="psC", bufs=2, space="PSUM") as psC,
        ):
            # ---- persistent SBUF tensors -------------------------------
            xqk = big.tile([128, CT, N], FP16, name="xqk", tag="xqk")
            wqk = big.tile([128, 12, CT, 128], FP16, name="wqk", tag="wqk")
            wvs = big.tile([128, CT, VW], FP16, name="wvs", tag="wvs")
            wps = big.tile([128, CT, C], FP16, name="wps", tag="wps")
            QT = big.tile([128, CT, N], FP16, name="QT", tag="QT")
            KT = big.tile([128, CT, N], FP16, name="KT", tag="KT")
            V = big.tile([128, NT, VW], FP16, name="V", tag="V")
            ctxN = big.tile([128, CT, N], FP16, name="ctxN", tag="ctxN")
            cst = big.tile([1, VW + C + 128], FP16, name="cst", tag="cst")
            ones_mask = [
                big.tile([1, 128], F32, name=f"ones_mask{i}", tag=f"onesr{i}")
                for i in range(2)
            ]
            vmask = cst[:, 0:VW]
            bias = cst[:, VW:VW + C]
            ones128 = cst[:, VW + C:VW + C + 128]

            # ---- input DMAs: few, large, in need-order ------------------
            nc.gpsimd.dma_start(cst[:], cst_d[:])
            for i in range(2):
                nc.gpsimd.dma_start(_r(ones_mask[i][:]), _r(onesr_d[i:i + 1, :]))
            nc.sync.dma_start(
                xqk[:].rearrange("p c n -> p (c n)"), xh_d[:]
            )
            wqk_f = wqk[:].rearrange("p s c j -> p (s c j)")
            nc.sync.dma_start(wqk_f[:, 0:2 * C], wqk_d[:, 0:2 * C])
            nc.sync.dma_start(
                wvs[:].rearrange("p c v -> p (c v)"), wv_d[:]
            )
            nc.sync.dma_start(wqk_f[:, 2 * C:12 * C], wqk_d[:, 2 * C:12 * C])
            nc.sync.dma_start(
                wps[:].rearrange("p c o -> p (c o)"), wp_d[:]
            )

            # ---- work-block emitters -----------------------------------
            def emit_c_slot(p, qk):
                # Q (qk=0) / K (qk=1) projection for pair p: d-major [128, N]
                s = p * 2 + qk
                dst = QT if qk == 0 else KT
                ps = psA.tile([128, N], F32, tag="ps", name=f"c{s}")
                for ct in range(CT):
                    for qc in range(2):
                        nc.tensor.matmul(
                            ps[:, qc * 512:(qc + 1) * 512],
                            wqk[:, s, ct, :],
                            xqk[:, ct, qc * 512:(qc + 1) * 512],
                            start=(ct == 0),
                            stop=(ct == CT - 1),
                        )
                nc.vector.tensor_copy(dst[:, p, :], ps[:])

            def emit_b_tile(nt):
                # V token-tile nt: [128 tok, 780] incl. denominator ones col
                pv = psA.tile([128, N], F32, tag="ps", name=f"b{nt}")
                for ct in range(CT):
                    for lo, w in ((0, 512), (512, VW - 512)):
                        nc.tensor.matmul(
                            pv[:, lo:lo + w],
                            xqk[:, ct, nt * 128:(nt + 1) * 128],
                            wvs[:, ct, lo:lo + w],
                            start=(ct == 0),
                            stop=False,
                        )
                for lo, w in ((0, 512), (512, VW - 512)):
                    nc.tensor.matmul(
                        pv[:, lo:lo + w],
                        ones128,
                        vmask[:, lo:lo + w],
                        start=False,
                        stop=True,
                    )
                nc.vector.tensor_copy(V[:, nt, :], pv[:, 0:VW])

            fillers = []
            for p, qk in ((1, 0), (1, 1), (2, 0), (2, 1)):
                fillers.append((emit_c_slot, (p, qk)))
            for nt in range(4):
                fillers.append((emit_b_tile, (nt,)))
            for nt in range(4, NT):
                fillers.append((emit_b_tile, (nt,)))
            for p, qk in ((3, 0), (3, 1), (4, 0), (4, 1), (5, 0), (5, 1)):
                fillers.append((emit_c_slot, (p, qk)))
            fillers.reverse()  # pop() from the end

            # ---- attention pipeline helpers ----------------------------
            deferred_norm = []

            def emit_norm(jobs):
                for i in range(0, len(jobs), 2):
                    emit_norm_pair(jobs[i:i + 2])

            def emit_norm_pair(jobs):
                p_ = jobs[0][2]
                rcrs = []
                for den_, h_, _p in jobs:
                    rc = normp.tile([1, N], F32, tag="rc", name=f"rc{h_}", bufs=2)
                    nc.vector.reciprocal_approx_fast(rc[:], den_[:])
                    rcr = normp.tile([1, N], F32, tag="rcr", name=f"rcr{h_}", bufs=2)
                    nc.vector.tensor_copy(_r(rcr[:]), rc[:])
                    rcrs.append(rcr)
                bc_ps = psA.tile([128, N], F32, tag="ps", name=f"bcp{p_}")
                for qc in range(2):
                    for half, rcr in enumerate(rcrs):
                        nc.tensor.matmul(
                            bc_ps[:, qc * 512:(qc + 1) * 512],
                            _r(ones_mask[half][:]),
                            _r(rcr[:, qc * 512:(qc + 1) * 512]),
                            start=(half == 0),
                            stop=(half == len(rcrs) - 1),
                        )
                bc = normp.tile([128, N], F32, tag="bc", name=f"bc{p_}", bufs=1)
                nc.vector.tensor_copy(bc[:], bc_ps[:])
                nc.vector.tensor_mul(ctxN[:, p_, :], ctxN[:, p_, :], bc[:])

            def emit_pv(pcps, pes, pp, kt):
                for half in range(2):
                    h = 2 * pp + half
                    for qc in range(2):
                        nc.tensor.matmul(
                            pcps[half][:, qc * 512:(qc + 1) * 512],
                            V[:, kt, h * DE:(h + 1) * DE],
                            pes[kt][half][:, qc * 512:(qc + 1) * 512],
                            start=(kt == 0),
                            stop=(kt == NT - 1),
                        )

            def emit_evac(pcps, pp):
                for half in range(2):
                    po = half * 64
                    nc.vector.tensor_copy(
                        ctxN[po:po + 64, pp, :], pcps[half][0:D, :]
                    )
                for half in range(2):
                    h = 2 * pp + half
                    den = normp.tile([1, N], F32, tag="den", name=f"den{h}")
                    nc.vector.tensor_copy(den[:], pcps[half][D:D + 1, :])
                    deferred_norm.append((den, h, pp))

            # ---- prologue: pair-0 Q/K ----------------------------------
            emit_c_slot(0, 0)
            emit_c_slot(0, 1)

            # ---- main pipelined streams --------------------------------
            prev = None
            for p in range(CT):
                cps = [
                    psC.tile([DE, N], F32, tag="ctx", name=f"ctx{2 * p + i}")
                    for i in range(2)
                ]
                es = []
                for kt in range(NT):
                    sps = [
                        psA.tile([128, N], F32, tag="ps", name=f"s{2 * p + i}_{kt}")
                        for i in range(2)
                    ]
                    for half in range(2):
                        po = half * 64
                        for qc in range(2):
                            nc.tensor.matmul(
                                sps[half][:, qc * 512:(qc + 1) * 512],
                                KT[po:po + 64, p, kt * 128:(kt + 1) * 128],
                                QT[po:po + 64, p, qc * 512:(qc + 1) * 512],
                                start=True,
                                stop=True,
                                tile_position=(po, 0),
                            )
                    row = []
                    for half in range(2):
                        h = 2 * p + half
                        e = ep.tile(
                            [128, N], FP16, tag="e", name=f"e{h}_{kt}", bufs=14
                        )
                        nc.scalar.activation(e[:], sps[half][:], EXP, scale=SCALE)
                        row.append(e)
                    es.append(row)
                    if prev is not None:
                        emit_pv(prev[0], prev[1], prev[2], kt)
                    if kt == 1 and deferred_norm:
                        emit_norm(deferred_norm)
                        deferred_norm = []
                    if fillers:
                        fn, args = fillers.pop()
                        fn(*args)
                if prev is not None:
                    emit_evac(prev[0], prev[2])
                prev = (cps, es, p)
            # drain: PV + evac for the final pair
            for kt in range(NT):
                emit_pv(prev[0], prev[1], prev[2], kt)
                if kt == 1 and deferred_norm:
                    emit_norm(deferred_norm)
                    deferred_norm = []
            emit_evac(prev[0], prev[2])
            emit_norm(deferred_norm)
            deferred_norm = []

            # ---- out-projection (+bias via K=1 ones-matmul) ------------
            for nt in range(NT):
                ps = psA.tile([128, N], F32, tag="ps", name=f"po{nt}")
                for lo, w in ((0, 512), (512, 256)):
                    for ct in range(CT):
                        nc.tensor.matmul(
                            ps[:, lo:lo + w],
                            ctxN[:, ct, nt * 128:(nt + 1) * 128],
                            wps[:, ct, lo:lo + w],
                            start=(ct == 0),
                            stop=False,
                        )
                    nc.tensor.matmul(
                        ps[:, lo:lo + w],
                        ones128,
                        bias[:, lo:lo + w],
                        start=False,
                        stop=True,
                    )
                ob = outp.tile([128, C], FP16, tag="ob", name=f"ob{nt}")
                nc.vector.tensor_copy(ob[:], ps[:, 0:C])
                nc.sync.dma_start(out_d[nt * 128:(nt + 1) * 128, :], ob[:])

    nc.compile()
    return nc


_CACHE = {}


def _get_nc():
    if "nc" not in _CACHE:
        _CACHE["nc"] = build()
    return _CACHE["nc"]


def _prep_maps(x, w_qkv, w_proj, b_proj):
    xh = np.ascontiguousarray(
        x.transpose(0, 2, 1)  # [B, C, N]
        .reshape(N_CORES, CT, 128, N)
        .transpose(0, 2, 1, 3)  # [B, 128, CT, N]
        .reshape(N_CORES, 128, CT * N)
    ).astype(np.float16)

    # wqk: slot s = jt*2+qk -> [128p, s, ct, j]
    wqk = np.empty((128, 12, CT, 128), dtype=np.float16)
    for jt in range(CT):
        for qk in range(2):
            blk = w_qkv[qk * C + jt * 128:qk * C + (jt + 1) * 128, :]  # [j, in]
            # [p, ct, j] = blk[j, ct*128+p]
            wqk[:, jt * 2 + qk] = (
                blk.T.reshape(CT, 128, 128).transpose(1, 0, 2)
            ).astype(np.float16)
    wqk = wqk.reshape(128, 12 * C)

    wv = w_qkv[2 * C:3 * C, :]  # [768 out, 768 in]
    wv_ext = np.zeros((H, DE, C), dtype=np.float64)
    wv_ext[:, 0:D, :] = wv.reshape(H, D, C)
    wv_ext = wv_ext.reshape(VW, C)  # [m, in]
    wvh = (
        wv_ext.T.reshape(CT, 128, VW).transpose(1, 0, 2).reshape(128, CT * VW)
    ).astype(np.float16)

    wph = (
        w_proj.T.reshape(CT, 128, C).transpose(1, 0, 2).reshape(128, CT * C)
    ).astype(np.float16)

    cst = np.zeros((1, VW + C + 128), dtype=np.float16)
    for h in range(H):
        cst[0, h * DE + D] = 1.0
    cst[0, VW:VW + C] = b_proj.astype(np.float16)
    cst[0, VW + C:] = 1.0

    onesr = np.kron(np.eye(2), np.ones((1, 64))).astype(np.float32)

    return [
        {
            "xh": xh[b],
            "wqk": wqk,
            "wv": wvh,
            "wp": wph,
            "cst": cst,
            "ones_mask": onesr,
        }
        for b in range(N_CORES)
    ]


def run(inputs, trace=False):
    """Run on hardware; returns (full output [8,1024,768] f32, results)."""
    nc = _get_nc()
    x = np.asarray(inputs["x"], dtype=np.float32)
    w_qkv = np.asarray(inputs["w_qkv"], dtype=np.float32)
    w_proj = np.asarray(inputs["w_proj"], dtype=np.float32)
    b_proj = np.asarray(inputs["b_proj"], dtype=np.float32)

    in_maps = _prep_maps(x, w_qkv, w_proj, b_proj)
    res = run_bass_kernel_spmd(nc, in_maps, list(range(N_CORES)), trace=trace)
    out = np.stack(
        [res.results[b]["out"].astype(np.float32) for b in range(N_CORES)]
    )
    return out, res


def kernel(x, w_qkv, w_proj, b_proj):
    out, _ = run(
        {"x": x, "w_qkv": w_qkv, "w_proj": w_proj, "b_proj": b_proj}, trace=False
    )
    return out
